# revision 10
# baseline (speedup 1.0000x reference)
"""DenseCapsule routing (2 iterations) on 8 Trainium2 cores.

Sharding: caps_in (C=2048) split across 8 cores (256 each); W-shard +
x-shard stay resident in SBUF, u is recomputed on the fly per c-tile.
Routing state is reduced across cores with two 128KB AllReduces.

Math (ITERATIONS=2, v0=0 => logits after iter1 are 0, cc1 = 1/K):
  u[b,k,c,i]   = sum_j W[k,c,i,j] x[b,c,j]
  v1           = squash(sum_c u / K)
  a[b,k,c]     = sum_i u[b,k,c,i] v1[b,k,i]        (logits for iter 2)
  cc           = softmax_k(a)
  v2           = squash(sum_c cc[b,k,c] u[b,k,c,i])   -> output

u / s1 / s2 / v1 live in (i, k) free-dim order (i outer, k inner) so
that every DVE broadcast (cc over i, g0 over i) has a packed innermost
k run and qualifies for the 2x perf mode.

Per-core layouts (host-prepped):
  xt  [(c,j)=2048, b=64]            pass-1 lhsT
  wt  [(c,j)=2048, (i,k)=512]       pass-1 rhs & pass-2 u-matmul rhs
  xdo [g=16, (c'16,j8)=128, oct=8, (c16,b8)=128]
      block-diag x: xdo[g,(c'j),o,(c,b)] = x[o*8+b, c0+16g+c', j] * (c==c')
      pass-2 u-matmul lhsT -> psum_u[(c,b), (i,k)] = u[b,k,c,i]
  obd [(c16,b'8)=128, oct=8, b=64]  ones block-diag: delta(b == o*8+b')
      s2p reduction lhsT: psum_s2[b,(i,k)] += sum_c tmp2[(c,b'),(i,k)]
"""

import numpy as np

import concourse.bacc as bacc
import concourse.bass as bass
import concourse.tile as tile
from concourse import mybir
from concourse._compat import with_exitstack
from concourse.bass_utils import run_bass_kernel_spmd

NC = 8
B = 64
C = 2048
J = 8
K = 32
I = 16
CL = C // NC        # 256 local caps_in
G = CL // 16        # 16 c-tiles (16 c's each -> 128 (c,j) rows)
KI = K * I          # 512
EPS = 1e-7

F32 = mybir.dt.float32
BF16 = mybir.dt.bfloat16

TRACE = False           # test.py sets True to capture NTFF timing
LAST_RESULTS = None     # BassKernelResults of the last run


def _bcast_last(ap, n):
    """Append a stride-0 dim of size n to an AP (free-dim broadcast)."""
    return bass.AP(tensor=ap.tensor, offset=ap.offset, ap=[*ap.ap, [0, n]])


def _bcast_ins(ap, n):
    """Insert a stride-0 dim of size n BEFORE the last free dim, keeping
    the innermost run packed (enables the DVE 2x perf mode)."""
    return bass.AP(tensor=ap.tensor, offset=ap.offset,
                   ap=[*ap.ap[:-1], [0, n], ap.ap[-1]])


def _squash(nc, pool, eps_t, s_sb, pre, out_dt=F32):
    """v = squash(pre * s_sb) for s_sb [B, (i,k)] f32, squash over i.

    squash(s) = (|s|^2 / (1 + |s|^2)) * s / sqrt(|s|^2 + EPS), per (b, k).
    Returns [B, I, K] tile of out_dt.
    """
    s3 = s_sb[:].rearrange("p (i k) -> p i k", k=K)
    sq = pool.tile([B, I, K], F32, tag="sq_sq")
    nc.vector.tensor_mul(sq[:], s3, s3)
    t1 = pool.tile([B, 8, K], F32, tag="sq_t1")
    nc.vector.tensor_add(t1[:], sq[:, 0:8, :], sq[:, 8:16, :])
    t2 = pool.tile([B, 4, K], F32, tag="sq_t2")
    nc.vector.tensor_add(t2[:], t1[:, 0:4, :], t1[:, 4:8, :])
    t3 = pool.tile([B, 2, K], F32, tag="sq_t3")
    nc.vector.tensor_add(t3[:], t2[:, 0:2, :], t2[:, 2:4, :])
    n0 = pool.tile([B, K], F32, tag="sq_n0")
    nc.vector.tensor_add(n0[:], t3[:, 0, :], t3[:, 1, :])
    sn = pool.tile([B, K], F32, tag="sq_sn")
    nc.scalar.mul(sn[:], n0[:], pre * pre)          # |s|^2
    rt = pool.tile([B, K], F32, tag="sq_rt")
    nc.scalar.activation(rt[:], sn[:], mybir.ActivationFunctionType.Sqrt,
                         bias=eps_t[:], scale=1.0)  # sqrt(|s|^2 + eps)
    dn = pool.tile([B, K], F32, tag="sq_dn")
    nc.scalar.add(dn[:], sn[:], 1.0)                # 1 + |s|^2
    dd = pool.tile([B, K], F32, tag="sq_dd")
    nc.vector.tensor_mul(dd[:], dn[:], rt[:])
    rc = pool.tile([B, K], F32, tag="sq_rc")
    nc.vector.reciprocal(rc[:], dd[:])
    f0 = pool.tile([B, K], F32, tag="sq_f0")
    nc.vector.tensor_mul(f0[:], sn[:], rc[:])
    g0 = pool.tile([B, K], F32, tag="sq_g0")
    nc.scalar.mul(g0[:], f0[:], pre)                # scale applied to raw s_sb
    v = pool.tile([B, I, K], out_dt, tag="sq_v")
    nc.vector.tensor_mul(v[:], s3, _bcast_ins(g0[:], I))
    return v


@with_exitstack
def _body(ctx, tc, xt, wt, xdo, obd, out_d):
    nc = tc.nc
    singles = ctx.enter_context(tc.tile_pool(name="singles", bufs=1))
    psA = ctx.enter_context(tc.tile_pool(name="psA", bufs=1, space="PSUM"))
    psU = ctx.enter_context(tc.tile_pool(name="psU", bufs=6, space="PSUM"))
    work = ctx.enter_context(tc.tile_pool(name="work", bufs=2))
    upool = ctx.enter_context(tc.tile_pool(name="upool", bufs=8))
    sm = ctx.enter_context(tc.tile_pool(name="sm", bufs=2))
    dram = ctx.enter_context(tc.tile_pool(name="dram", bufs=1, space="DRAM"))
    ar1_in = dram.tile([B, KI], BF16, name="ar1_in")
    ar1_out = dram.tile([B, KI], BF16, name="ar1_out", addr_space="Shared")
    ar2_in = dram.tile([B, KI], BF16, name="ar2_in")
    ar2_out = dram.tile([B, KI], BF16, name="ar2_out", addr_space="Shared")

    # ---- resident inputs (one tile per DMA so consumers wait on 1 sem) ----
    xt_sb = [singles.tile([128, B], BF16, name=f"xt{g}", tag=f"xt{g}") for g in range(G)]
    wt_sb = [singles.tile([128, KI], BF16, name=f"wt{g}", tag=f"wt{g}") for g in range(G)]
    xdo_sb = [singles.tile([128, 8, 128], BF16, name=f"xdo{g}", tag=f"xdo{g}") for g in range(G)]
    obd_sb = singles.tile([128, 8, B], BF16)
    for g in range(G):
        nc.sync.dma_start(out=xt_sb[g][:], in_=xt[g * 128:(g + 1) * 128, :])
        nc.sync.dma_start(out=wt_sb[g][:], in_=wt[g * 128:(g + 1) * 128, :])
    nc.sync.dma_start(out=obd_sb[:], in_=obd)
    for g in range(G):
        nc.sync.dma_start(out=xdo_sb[g][:], in_=xdo[g])
    eps_t = singles.tile([B, 1], F32)
    nc.vector.memset(eps_t[:], EPS)

    # ---- pass 1: s1 partial = sum_{c local, j} W x ----
    ps_s1 = psA.tile([B, KI], F32)
    for g in range(G):
        nc.tensor.matmul(ps_s1[:], lhsT=xt_sb[g][:],
                         rhs=wt_sb[g][:],
                         start=(g == 0), stop=(g == G - 1))
    # ---- pass 2, software-pipelined: produce u tiles (PE+ACT), consume
    # (DVE routing chain) once v1 is ready.  PRO tiles are produced before
    # the AllReduce so PE/ACT fill the collective latency.
    ps_s2 = psA.tile([B, KI], F32)
    nmm = 8 * G
    it = 0
    PRO = 7
    u_tiles = {}

    def produce(g):
        u_g = upool.tile([128, 8, KI], BF16, name=f"ug{g}", tag="ug")
        for o in range(8):
            ps_u = psU.tile([128, KI], F32, tag="psu")
            nc.tensor.matmul(ps_u[:], lhsT=xdo_sb[g][:, o, :],
                             rhs=wt_sb[g][:],
                             start=True, stop=True)
            nc.scalar.copy(u_g[:, o, :], ps_u[:])
        u_tiles[g] = u_g

    def consume(g):
        nonlocal it
        u_g = u_tiles.pop(g)
        tmp = work.tile([128, 8, KI], BF16, tag="tmp")
        nc.vector.tensor_mul(tmp[:], u_g[:], v1rep[:])
        t4 = tmp[:].rearrange("p o (i k) -> p o i k", k=K)
        f1 = work.tile([128, 8, 8, K], BF16, tag="f1")
        nc.vector.tensor_add(f1[:], t4[:, :, 0:8, :], t4[:, :, 8:16, :])
        f2 = sm.tile([128, 8, 4, K], BF16, tag="f2")
        nc.vector.tensor_add(f2[:], f1[:, :, 0:4, :], f1[:, :, 4:8, :])
        f3 = sm.tile([128, 8, 2, K], BF16, tag="f3")
        nc.vector.tensor_add(f3[:], f2[:, :, 0:2, :], f2[:, :, 2:4, :])
        a_t = sm.tile([128, 8, K], F32, tag="a")
        nc.vector.tensor_add(a_t[:], f3[:, :, 0, :], f3[:, :, 1, :])
        e_t = sm.tile([128, 8, K], F32, tag="e")
        nc.scalar.activation(e_t[:], a_t[:],
                             mybir.ActivationFunctionType.Exp, scale=1.0)
        den = sm.tile([128, 8], F32, tag="den")
        nc.vector.reduce_sum(den[:], e_t[:], axis=mybir.AxisListType.X)
        rcp = sm.tile([128, 8], F32, tag="rcp")
        nc.vector.reciprocal(rcp[:], den[:])
        cc = sm.tile([128, 8, K], BF16, tag="cc")
        nc.vector.tensor_mul(cc[:], e_t[:], _bcast_last(rcp[:], K))
        tmp2 = work.tile([128, 8, I, K], BF16, tag="tmp2")
        nc.vector.tensor_mul(
            tmp2[:], u_g[:].rearrange("p o (i k) -> p o i k", k=K),
            _bcast_ins(cc[:], I))
        for o in range(8):
            nc.tensor.matmul(ps_s2[:], lhsT=obd_sb[:, o, :],
                             rhs=tmp2[:, o, :, :].rearrange("p i k -> p (i k)"),
                             start=(it == 0), stop=(it == nmm - 1))
            it += 1

    for g in range(PRO):
        produce(g)

    # ---- AllReduce s1, v1 = squash(s1/K), replicate across partitions ----
    s1p = sm.tile([B, KI], BF16, tag="s1p")
    nc.scalar.copy(s1p[:], ps_s1[:])
    nc.sync.dma_start(out=ar1_in[:], in_=s1p[:])
    nc.gpsimd.collective_compute(
        "AllReduce", mybir.AluOpType.add,
        replica_groups=[list(range(NC))], ins=[ar1_in.opt()], outs=[ar1_out.opt()])
    s1 = sm.tile([B, KI], BF16, tag="s1")
    nc.sync.dma_start(out=s1[:], in_=ar1_out[:])
    v1 = _squash(nc, sm, eps_t, s1, 1.0 / K)
    v1b = sm.tile([B, KI], BF16, tag="v1b")
    nc.vector.tensor_copy(v1b[:], v1[:].rearrange("p i k -> p (i k)"))
    v1rep = singles.tile([128, 8, KI], BF16)
    v1d = dram.tile([B, KI], BF16, name="v1d")
    nc.sync.dma_start(out=v1d[:], in_=v1b[:])
    v1d_ap = v1d[:]
    for o in range(8):
        src_ap = bass.AP(tensor=v1d_ap.tensor,
                         offset=v1d_ap.offset + o * 8 * KI,
                         ap=[[0, 16], [KI, 8], [1, KI]])
        nc.sync.dma_start(out=v1rep[:, o, :], in_=src_ap)

    for g in range(PRO, G):
        produce(g)
        consume(g - PRO)
    for g in range(G - PRO, G):
        consume(g)

    s2p = sm.tile([B, KI], BF16, tag="s2p")
    nc.scalar.copy(s2p[:], ps_s2[:])
    nc.sync.dma_start(out=ar2_in[:], in_=s2p[:])
    nc.gpsimd.collective_compute(
        "AllReduce", mybir.AluOpType.add,
        replica_groups=[list(range(NC))], ins=[ar2_in.opt()], outs=[ar2_out.opt()])
    s2 = sm.tile([B, KI], BF16, tag="s2")
    nc.sync.dma_start(out=s2[:], in_=ar2_out[:])
    v2 = _squash(nc, sm, eps_t, s2, 1.0)
    nc.sync.dma_start(out=out_d, in_=v2[:].rearrange("p i k -> p (i k)"))


_PROG = None


def _get_program():
    global _PROG
    if _PROG is None:
        nc = bacc.Bacc("TRN2", target_bir_lowering=False, debug=False,
                       num_devices=NC)
        xt_d = nc.dram_tensor("xt", [CL * J, B], BF16, kind="ExternalInput")
        wt_d = nc.dram_tensor("wt", [CL * J, KI], BF16, kind="ExternalInput")
        xdo_d = nc.dram_tensor("xdo", [G, 128, 8, 128], BF16,
                               kind="ExternalInput")
        obd_d = nc.dram_tensor("obd", [128, 8, B], BF16, kind="ExternalInput")
        out_d = nc.dram_tensor("out", [B, KI], F32, kind="ExternalOutput")
        with tile.TileContext(nc) as tc:
            _body(tc, xt_d[:], wt_d[:], xdo_d[:], obd_d[:], out_d[:])
        nc.compile()
        _PROG = nc
    return _PROG


def _constant_mats():
    import ml_dtypes
    obd = np.zeros((16, 8, 8, B), np.float32)       # [c, b', oct, b]
    for o in range(8):
        for bp in range(8):
            obd[:, bp, o, o * 8 + bp] = 1.0
    obd = obd.reshape(128, 8, B).astype(ml_dtypes.bfloat16)
    return obd


def kernel(x, W):
    global LAST_RESULTS
    x = np.ascontiguousarray(np.asarray(x, np.float32))
    W = np.ascontiguousarray(np.asarray(W, np.float32))
    assert x.shape == (B, C, J) and W.shape == (K, C, I, J)
    nc = _get_program()
    obd = _constant_mats()
    in_maps = []
    for m in range(NC):
        xs = x[:, m * CL:(m + 1) * CL, :]                       # [B, CL, J]
        Ws = W[:, m * CL:(m + 1) * CL, :, :]                    # [K, CL, I, J]
        import ml_dtypes
        bf = ml_dtypes.bfloat16
        xt = np.ascontiguousarray(
            xs.transpose(1, 2, 0)).reshape(CL * J, B).astype(bf)
        wt = np.ascontiguousarray(
            Ws.transpose(1, 3, 2, 0)).reshape(CL * J, KI).astype(bf)
        A = xs.reshape(8, 8, G, 16, J)                          # [o, b, g, c', j]
        xdo = np.zeros((G, 16, J, 8, 16, 8), np.float32)        # [g,c',j,o,c,b]
        for cp in range(16):
            xdo[:, cp, :, :, cp, :] = A[:, :, :, cp, :].transpose(2, 3, 0, 1)
        xdo = xdo.reshape(G, 128, 8, 128).astype(bf)
        in_maps.append({"xt": xt, "wt": wt, "xdo": xdo, "obd": obd})
    res = run_bass_kernel_spmd(nc, in_maps, core_ids=list(range(NC)),
                               trace=TRACE)
    LAST_RESULTS = res
    out = np.asarray(res.results[0]["out"], np.float32).reshape(B, I, K)
    return np.ascontiguousarray(out.transpose(0, 2, 1))         # [B, K, I]
